# revision 1
# baseline (speedup 1.0000x reference)
"""Bass/Tile kernel for causal self-attention, head-sharded across cores.

Per-core layout (core c owns heads 2c, 2c+1):
  inputs (per core):
    xT    [C, B*T]        bf16   x transposed (feature-major), same on all cores
    wqkv  [128, KC, F]    bf16   W_qkv column-slice, [p, kchunk, f]; f = [q_h0|q_h1|k_h0|k_h1|v_h0|v_h1] * 64
    wproj [128, C]        bf16   W_proj row-slice (rows = this core's 128 head dims)
    bqkv  [128, FC]       f32    b_qkv slice, partition-major per f-chunk
    pbias [128, B, T/128] f32    key-padding bias (0 or -1e30), partition-major per key chunk
    mkd   [128, CPB, 2, 128] bf16  0/1 causal masks for the partially-masked
                                 128-col strip of each diagonal tile, both heads
    bvec  [128, CPB, 128] bf16   V-bias replicated across partitions/chunks
  output:
    outT  [C, B*T]        bf16   partial projection output (pre-bias), feature-major

V is produced token-major directly on the PE (x chunk stationary, V-weights
moving) so no DMA transpose is needed.  S^T/PV/denominator matmuls and the
softmax exp skip the fully-masked query columns of diagonal key chunks.

The PE instruction order is fully explicit: a software-pipelined stream
  ... S(iter i) | one filler unit (QKV psum-group or proj half-block) | PV/denom(iter i-1) ...
enforced with ordering-only dependency edges between consecutive PE
instructions.  The softmax exp for iter i-1 runs on ACT while S(i) and the
filler stream on the PE, so PV(i-1) never stalls and the ACT engine is fed
continuously from ~20us onward.  Mask multiplies run on the (otherwise idle)
GpSimd engine to keep DVE free for bias-adds/normalize/proj copies.
"""

import concourse.bass as bass
import concourse.mybir as mybir
import concourse.tile as tile
from concourse import bacc
from concourse.tile import add_dep_helper

F32 = mybir.dt.float32
BF16 = mybir.dt.bfloat16
AF = mybir.ActivationFunctionType


def build_nc(B=4, T=2048, C=1024, HPC=2, D=64, TB=512, num_devices=8,
             scale=None, out_dtype=BF16):
    if scale is None:
        scale = D ** -0.5
    NT = B * T                 # total tokens
    NB = NT // TB              # 512-token blocks (global)
    BPB = T // TB              # blocks per batch
    CPB = TB // 128            # 128-chunks per block (4)
    NCH = T // 128             # key chunks per batch
    KC = C // 128              # contraction chunks for qkv matmul
    F = HPC * 3 * D            # per-core qkv features (384)
    FC = F // 128              # f-chunks (3)
    FCO = C // 128             # proj output feature chunks (8)
    GRP = BPB                  # qkv group == one batch
    assert HPC == 2 and HPC * D == 128 and F % 128 == 0 and TB % 128 == 0

    nc = bacc.Bacc("TRN2", target_bir_lowering=False, debug=False,
                   num_devices=num_devices)

    xT = nc.dram_tensor("xT", [C, NT], BF16, kind="ExternalInput")
    wqkv = nc.dram_tensor("wqkv", [128, KC, F], BF16, kind="ExternalInput")
    wproj = nc.dram_tensor("wproj", [128, C], BF16, kind="ExternalInput")
    bqkv = nc.dram_tensor("bqkv", [128, FC], F32, kind="ExternalInput")
    pbias = nc.dram_tensor("pbias", [128, B, NCH], F32, kind="ExternalInput")
    mkd = nc.dram_tensor("mkd", [128, CPB, 2, 128], BF16, kind="ExternalInput")
    bvec = nc.dram_tensor("bvec", [128, CPB, 128], BF16, kind="ExternalInput")
    outT = nc.dram_tensor("outT", [C, NT], out_dtype, kind="ExternalOutput")
    outT_r = outT.rearrange("(f p) t -> p f t", p=128)

    with tile.TileContext(nc) as tc:
        with (
            tc.tile_pool(name="const", bufs=1) as const,
            tc.tile_pool(name="persist", bufs=1) as persist,
            tc.tile_pool(name="xp", bufs=10) as xp,
            tc.tile_pool(name="pp", bufs=10) as pp,
            tc.tile_pool(name="rp", bufs=2) as rp,
            tc.tile_pool(name="op", bufs=2) as op,
            tc.tile_pool(name="psmm", bufs=2, space="PSUM") as psmm,
            tc.tile_pool(name="psa", bufs=2, space="PSUM") as psa,
            tc.tile_pool(name="pss", bufs=2, space="PSUM") as pss,
        ):
            # ---- constants ----
            # ring order matters: small early-needed consts first, then
            # group-0's second x half, then late-needed weights (mk/wp are
            # emitted after the x loads in push_qkv_group(0))
            w_sb = const.tile([128, KC, F], BF16, tag="w", name="w_sb")
            nc.gpsimd.dma_start(out=w_sb[:], in_=wqkv[:])
            bq_sb = const.tile([128, FC], F32, tag="bq", name="bq_sb")
            nc.gpsimd.dma_start(out=bq_sb[:], in_=bqkv[:])
            pb_sb = const.tile([128, B, NCH], F32, tag="pb", name="pb_sb")
            nc.gpsimd.dma_start(out=pb_sb[:], in_=pbias[:])
            bv_sb = const.tile([128, CPB, 128], BF16, tag="bv", name="bv_sb")
            nc.gpsimd.dma_start(out=bv_sb[:], in_=bvec[:])
            wp_sb = const.tile([128, C], BF16, tag="wp", name="wp_sb")
            mk_sb = const.tile([128, CPB, 2, 128], BF16, tag="mk", name="mk_sb")
            ones_sb = const.tile([128, 64], BF16, tag="ones", name="ones_sb")
            nc.vector.memset(ones_sb[:], 1.0)

            def load_late_consts():
                nc.gpsimd.dma_start(out=mk_sb[:], in_=mkd[:])
                nc.gpsimd.dma_start(out=wp_sb[:], in_=wproj[:])

            # ---- persistent per-block tiles ----
            qT = [persist.tile([128, TB], BF16, tag=f"qT{i}", name=f"qT{i}")
                  for i in range(NB)]
            kT = [persist.tile([128, TB], BF16, tag=f"kT{i}", name=f"kT{i}")
                  for i in range(NB)]
            V = [persist.tile([128, CPB, 128], BF16, tag=f"V{i}", name=f"V{i}")
                 for i in range(NB)]

            # ---- explicit PE-order chain ----
            chain = [None]

            def pe(mm):
                if chain[0] is not None:
                    add_dep_helper(mm.ins, chain[0].ins, sync=False,
                                   reason="explicit PE order")
                chain[0] = mm
                return mm

            # ---- x loads ----
            x_of = {}

            def load_group_x(g):
                # group 0 split in halves on the sync HWDGE ring (fast
                # start); later groups wide on the gpsimd ring with 12
                # slots so group g+1's prefetch starts early
                W = GRP * TB
                if g == 0:
                    # half 0 on the sync ring, half 1 on the gpsimd ring
                    # (right after the small consts) so both halves of
                    # group 0's x stream in parallel at startup
                    for half in range(2):
                        row = []
                        eng = nc.sync if half == 0 else nc.gpsimd
                        for kc in range(KC):
                            xt = xp.tile([128, W // 2], BF16, tag="xt",
                                         name="xt", bufs=16)
                            eng.dma_start(
                                out=xt[:],
                                in_=xT[kc * 128:(kc + 1) * 128,
                                       half * W // 2:(half + 1) * W // 2])
                            row.append(xt)
                        for tl in range(2):
                            for kc in range(KC):
                                x_of[(g * GRP + half * 2 + tl, kc)] = \
                                    row[kc][:, tl * TB:(tl + 1) * TB]
                    load_late_consts()
                else:
                    for kc in range(KC):
                        xt = xp.tile([128, W], BF16, tag="xtw", name="xtw",
                                     bufs=10)
                        nc.gpsimd.dma_start(
                            out=xt[:],
                            in_=xT[kc * 128:(kc + 1) * 128, g * W:(g + 1) * W])
                        for tl in range(GRP):
                            x_of[(g * GRP + tl, kc)] = \
                                xt[:, tl * TB:(tl + 1) * TB]

            # ---- filler units (emitted on demand, PE-chained) ----
            def qkv_unit(tb, fc):
                ps = psmm.tile([128, TB], F32, tag="ps", name="ps")
                for kc in range(KC):
                    mm = nc.tensor.matmul(
                        ps[:], lhsT=w_sb[:, kc, fc * 128:(fc + 1) * 128],
                        rhs=x_of[(tb, kc)],
                        start=(kc == 0), stop=(kc == KC - 1))
                    if kc == 0:
                        # chain only the unit head; the accumulation group
                        # orders the rest, leaving LDWEIGHTS free to pull
                        # ahead of in-flight matmuls
                        pe(mm)
                    else:
                        chain[0] = mm
                dest = qT[tb] if fc == 0 else kT[tb]
                nc.vector.tensor_scalar_add(
                    out=dest[:], in0=ps[:], scalar1=bq_sb[:, fc:fc + 1])

            def v_unit(tb):
                # V produced token-major directly: x chunk is the stationary
                # operand, V-weights stream; out[tokens, features] per chunk.
                # Avoids the DMA-transpose (xbar-mode serialization).
                psV = psmm.tile([128, CPB, 128], F32, tag="ps", name="psV")
                for cl in range(CPB):
                    for kc in range(KC):
                        mm = nc.tensor.matmul(
                            psV[:, cl, :],
                            lhsT=x_of[(tb, kc)][:, cl * 128:(cl + 1) * 128],
                            rhs=w_sb[:, kc, 2 * 128:3 * 128],
                            start=(kc == 0), stop=(kc == KC - 1),
                            skip_group_check=True)
                        if kc == 0:
                            pe(mm)
                        else:
                            chain[0] = mm
                nc.vector.tensor_add(V[tb][:], psV[:], bv_sb[:])

            def proj_unit(gb, at, stage, half, last=False):
                h0 = half * FCO // 2
                for fc in range(h0, h0 + FCO // 2):
                    # the final block's proj may also rotate through the
                    # freed psa slots (psO/psD are done) -> 4-deep pipeline
                    if last and fc % 2 == 1:
                        ps = psa.tile([128, TB], F32, tag="pa", name="psp")
                    else:
                        ps = psmm.tile([128, TB], F32, tag="ps", name="ps")
                    pe(nc.tensor.matmul(
                        ps[:], lhsT=wp_sb[:, fc * 128:(fc + 1) * 128],
                        rhs=at[:], start=True, stop=True))
                    nc.any.tensor_copy(stage[:, fc, :], ps[:])
                nc.gpsimd.dma_start(
                    out=outT_r[:, h0:h0 + FCO // 2, gb * TB:(gb + 1) * TB],
                    in_=stage[:, h0:h0 + FCO // 2, :])

            filler = []          # queue of (kind, payload) closures
            filler_pos = [0]     # how many qkv units of each group drained

            def drain_filler(n=None, through_qkv=None):
                # through_qkv: drain (incl. proj heads in the way) until every
                # qkv unit with tb <= through_qkv has been emitted
                while filler:
                    if n is not None and n <= 0:
                        break
                    kind, key, fn = filler[0]
                    if n is None and through_qkv is not None:
                        if kind == "qkv" and key > through_qkv:
                            break
                        if not any(k == "qkv" and ky <= through_qkv
                                   for k, ky, _ in filler):
                            break
                    filler.pop(0)
                    fn()
                    if n is not None:
                        n -= 1

            def push_qkv_group(g):
                load_group_x(g)
                for tl in range(GRP):
                    tb = g * GRP + tl
                    for fc in range(2):
                        filler.append(
                            ("qkv", tb,
                             lambda tb=tb, fc=fc: qkv_unit(tb, fc)))
                    filler.append(("qkv", tb, lambda tb=tb: v_unit(tb)))

            # ---- attention ----
            def s_iter(b, qb, c0, nchunks):
                """Emit S matmuls + exp (+mask) for chunks c0, c0+1."""
                gb = b * BPB + qb
                pts, offs = [], []
                for ci in (c0, c0 + 1):
                    cb, cl = divmod(ci, CPB)
                    ktile = kT[b * BPB + cb]
                    didx = ci - qb * CPB
                    off = didx * 128 if didx >= 0 else 0
                    offs.append(off)
                    psS = pss.tile([128, 2 * TB], F32, tag="pss", name="psS")
                    for h in range(HPC):
                        pe(nc.tensor.matmul(
                            psS[:, h * TB + off:(h + 1) * TB],
                            lhsT=ktile[h * 64:(h + 1) * 64,
                                       cl * 128:(cl + 1) * 128],
                            rhs=qT[gb][h * 64:(h + 1) * 64, off:],
                            start=True, stop=True,
                            tile_position=(h * 64, 0)))
                    pt = pp.tile([128, 2 * TB], BF16, tag="pt", name="pt")
                    psS3 = psS.rearrange("p (h t) -> p h t", h=2)
                    pt3 = pt.rearrange("p (h t) -> p h t", h=2)
                    nc.scalar.activation(
                        out=pt3[:, :, off:], in_=psS3[:, :, off:],
                        func=AF.Exp,
                        bias=pb_sb[:, b, ci:ci + 1], scale=scale)
                    if didx >= 0:
                        nc.gpsimd.tensor_tensor(
                            out=pt3[:, :, off:off + 128],
                            in0=pt3[:, :, off:off + 128],
                            in1=mk_sb[:, didx],
                            op=mybir.AluOpType.mult)
                    pts.append(pt)
                return pts, offs

            def pv_iter(b, qb, c0, nchunks, pts, offs, psO, psD):
                def emit_pv():
                    for j, ci in enumerate((c0, c0 + 1)):
                        vtile = V[b * BPB + ci // CPB]
                        off = offs[j]
                        for h in range(HPC):
                            pe(nc.tensor.matmul(
                                psO[h * 64:(h + 1) * 64, off:],
                                lhsT=vtile[:, ci % CPB, h * 64:(h + 1) * 64],
                                rhs=pts[j][:, h * TB + off:(h + 1) * TB],
                                start=(ci == 0), stop=(ci == nchunks - 1),
                                tile_position=(0, h * 64),
                                skip_group_check=True))

                def emit_denom():
                    for j, ci in enumerate((c0, c0 + 1)):
                        off = offs[j]
                        for h in range(HPC):
                            pe(nc.tensor.matmul(
                                psD[h * 64:(h + 1) * 64, off:],
                                lhsT=ones_sb[:],
                                rhs=pts[j][:, h * TB + off:(h + 1) * TB],
                                start=(ci == 0), stop=(ci == nchunks - 1),
                                tile_position=(0, h * 64),
                                skip_group_check=True))

                if c0 + 2 >= nchunks:
                    # last iteration of the block: denominators first so the
                    # reciprocal (gating at-mul -> proj) starts earlier
                    emit_denom()
                    emit_pv()
                else:
                    emit_pv()
                    emit_denom()

            def finish_block(b, qb, psO, psD, last=False):
                gb = b * BPB + qb
                rt = rp.tile([128, TB], F32, tag="rt", name="rt")
                nc.vector.reciprocal_approx_fast(out=rt[:], in_=psD[:])
                at = pp.tile([128, TB], BF16, tag="at", name="at", bufs=4)
                nc.vector.tensor_mul(at[:], psO[:], rt[:])
                stage = op.tile([128, FCO, TB], out_dtype, tag="st",
                                name="stage")
                filler.append(("proj", -1,
                               lambda: proj_unit(gb, at, stage, 0, last)))
                filler.append(("proj", -1,
                               lambda: proj_unit(gb, at, stage, 1, last)))

            # ---- main emission: software-pipelined global stream ----
            push_qkv_group(0)
            pend = None          # (b, qb, c0, nchunks, pts, offs, psO, psD)
            cur = None           # (psO, psD) of the block being accumulated

            for b in range(B):
                if b + 1 < B:
                    push_qkv_group(b + 1)
                for qb in range(BPB):
                    gb = b * BPB + qb
                    nchunks = (qb + 1) * CPB
                    # block gb needs qkv units of blocks <= gb drained
                    drain_filler(through_qkv=gb)
                    psO = psa.tile([128, TB], F32, tag="pa", name="psO")
                    psD = psa.tile([128, TB], F32, tag="pa", name="psD")
                    for c0 in range(0, nchunks, 2):
                        pts, offs = s_iter(b, qb, c0, nchunks)
                        drain_filler(n=2 if len(filler) > 10 else 1)
                        if pend is not None:
                            pv_iter(*pend)
                            if pend[2] + 2 >= pend[3]:   # last iter of block
                                finish_block(pend[0], pend[1], pend[6],
                                             pend[7])
                        pend = (b, qb, c0, nchunks, pts, offs, psO, psD)
            # flush the last pending iteration
            pv_iter(*pend)
            finish_block(pend[0], pend[1], pend[6], pend[7], last=True)
            drain_filler(n=len(filler))

    nc.compile()
    return nc


def prep_core_inputs(x, key_padding_mask, W_qkv, b_qkv, W_proj,
                     n_cores=8, TB=512):
    """Host-side sharding: build the per-core input maps."""
    import numpy as np
    import ml_dtypes

    B, T, C = x.shape
    D = 64
    H = C // D
    HPC = H // n_cores
    BT = B * T
    CPB = TB // 128

    xT = np.ascontiguousarray(
        x.reshape(BT, C).T).astype(ml_dtypes.bfloat16)          # [C, BT]

    pb = np.where(key_padding_mask, np.float32(-1e30),
                  np.float32(0.0)).astype(np.float32)           # [B, T]
    pb = np.ascontiguousarray(pb.reshape(B, T // 128, 128).transpose(2, 0, 1))

    # partial-mask strip for diagonal tiles: for offset idx, cols
    # [idx*128, idx*128+128) need mask (idx*128 + p <= j); replicated per head
    p = np.arange(128)[:, None]
    j = np.arange(128)[None, :]
    mk = np.stack([(p <= j) for _ in range(CPB)], axis=1)       # [128,CPB,128]
    mk = np.repeat(mk[:, :, None, :], 2, axis=2)                # [128,CPB,2,128]
    mk = mk.astype(ml_dtypes.bfloat16)

    KC = C // 128
    in_maps = []
    for c in range(n_cores):
        hs = [HPC * c + i for i in range(HPC)]
        cols = np.concatenate([
            np.concatenate([which * H * D + h * D + np.arange(D) for h in hs])
            for which in range(3)])                             # [F]
        Wc = W_qkv[:, cols]                                     # [C, F]
        F = Wc.shape[1]
        wq = np.ascontiguousarray(
            Wc.reshape(KC, 128, F).transpose(1, 0, 2)).astype(ml_dtypes.bfloat16)
        bq = np.ascontiguousarray(
            b_qkv[cols].reshape(F // 128, 128).T).astype(np.float32)
        rows = np.concatenate([h * D + np.arange(D) for h in hs])
        wp = np.ascontiguousarray(W_proj[rows, :]).astype(ml_dtypes.bfloat16)
        bv = b_qkv[cols][2 * 128:3 * 128]                       # v-bias [128]
        bvec = np.ascontiguousarray(
            np.broadcast_to(bv[None, None, :], (128, CPB, 128))
        ).astype(ml_dtypes.bfloat16)
        in_maps.append({
            "xT": xT, "wqkv": wq.reshape(128, KC, F), "wproj": wp,
            "bqkv": bq, "pbias": pb, "mkd": mk, "bvec": bvec,
        })
    return in_maps


def combine_outputs(results, B, T, C, b_proj):
    import numpy as np
    acc = results[0]["outT"].astype(np.float32)
    for r in results[1:]:
        acc = acc + r["outT"].astype(np.float32)
    out = acc.T.reshape(B, T, C) + b_proj.astype(np.float32)
    return out.astype(np.float32)


# ---------------------------------------------------------------------------
# Self-contained entry point for the grading harness.
# kernel(**inputs) takes the FULL unsharded inputs and returns the FULL output.
# Sharding: tensor-parallel over heads (2 heads per core, 8 cores); each core
# computes its QKV column-slice, attention for its heads, and a partial output
# projection; partials are summed on the host.
# ---------------------------------------------------------------------------
import numpy as np

_NC_CACHE = {}


def _get_nc():
    if "nc" not in _NC_CACHE:
        _NC_CACHE["nc"] = build_nc(B=4, T=2048, C=1024, num_devices=8)
    return _NC_CACHE["nc"]


def kernel(x, key_padding_mask, W_qkv, b_qkv, W_proj, b_proj):
    from concourse.bass_utils import run_bass_kernel_spmd

    x = np.asarray(x, dtype=np.float32)
    key_padding_mask = np.asarray(key_padding_mask).astype(bool)
    W_qkv = np.asarray(W_qkv, dtype=np.float32)
    b_qkv = np.asarray(b_qkv, dtype=np.float32)
    W_proj = np.asarray(W_proj, dtype=np.float32)
    b_proj = np.asarray(b_proj, dtype=np.float32)

    B, T, C = x.shape
    nc = _get_nc()
    in_maps = prep_core_inputs(x, key_padding_mask, W_qkv, b_qkv, W_proj,
                               n_cores=8)
    res = run_bass_kernel_spmd(nc, in_maps, list(range(8)))
    return combine_outputs(res.results, B, T, C, b_proj)



# revision 7
# speedup vs baseline: 1.1439x; 1.1439x over previous
"""Bass/Tile kernel for causal self-attention, head-sharded across cores.

Per-core layout (core c owns heads 2c, 2c+1):
  inputs (per core):
    xT    [C, B*T]        bf16   x transposed (feature-major), same on all cores
    wqkv  [128, KC, F]    bf16   W_qkv column-slice, [p, kchunk, f]; f = [q_h0|q_h1|k_h0|k_h1|v_h0|v_h1] * 64
    wproj [128, C]        bf16   W_proj row-slice (rows = this core's 128 head dims)
    bqkv  [128, FC]       f32    b_qkv slice, partition-major per f-chunk
    pbias [128, B, T/128] f32    key-padding bias (0 or -1e30), partition-major per key chunk
    mkd   [128, CPB, 2, 128] bf16  0/1 causal masks for the partially-masked
                                 128-col strip of each diagonal tile, both heads
    bvec  [128, CPB, 128] bf16   V-bias replicated across partitions/chunks
  output:
    outT  [C, B*T]        bf16   partial projection output (pre-bias), feature-major

V is produced token-major directly on the PE (x chunk stationary, V-weights
moving) so no DMA transpose is needed.  S^T/PV/denominator matmuls and the
softmax exp skip the fully-masked query columns of diagonal key chunks.

The PE instruction order is fully explicit: a software-pipelined stream
  ... S(iter i) | filler (QKV psum-group or single proj matmul) | PV/denom(iter i-2) ...
enforced with ordering-only dependency edges between consecutive PE
instructions.  PV lags S by TWO iterations so each ACT exp has ~2 iterations
of PE work to hide behind.  Proj matmuls are emitted one at a time as filler
so their PSUM->SBUF copies never back up against the 2 psmm banks.  Mask
multiplies run on the (otherwise idle) GpSimd engine to keep DVE free for
bias-adds/normalize/proj copies.
"""

import concourse.bass as bass
import concourse.mybir as mybir
import concourse.tile as tile
from concourse import bacc
from concourse.tile import add_dep_helper

F32 = mybir.dt.float32
BF16 = mybir.dt.bfloat16
AF = mybir.ActivationFunctionType


def build_nc(B=4, T=2048, C=1024, HPC=2, D=64, TB=512, num_devices=8,
             scale=None, out_dtype=BF16):
    if scale is None:
        scale = D ** -0.5
    NT = B * T                 # total tokens
    NB = NT // TB              # 512-token blocks (global)
    BPB = T // TB              # blocks per batch
    CPB = TB // 128            # 128-chunks per block (4)
    NCH = T // 128             # key chunks per batch
    KC = C // 128              # contraction chunks for qkv matmul
    F = HPC * 3 * D            # per-core qkv features (384)
    FC = F // 128              # f-chunks (3)
    FCO = C // 128             # proj output feature chunks (8)
    GRP = BPB                  # qkv group == one batch
    assert HPC == 2 and HPC * D == 128 and F % 128 == 0 and TB % 128 == 0

    nc = bacc.Bacc("TRN2", target_bir_lowering=False, debug=False,
                   num_devices=num_devices)

    xT = nc.dram_tensor("xT", [C, NT], BF16, kind="ExternalInput")
    wqkv = nc.dram_tensor("wqkv", [128, KC, F], BF16, kind="ExternalInput")
    wproj = nc.dram_tensor("wproj", [128, C], BF16, kind="ExternalInput")
    bqkv = nc.dram_tensor("bqkv", [128, FC], F32, kind="ExternalInput")
    pbias = nc.dram_tensor("pbias", [128, B, NCH], F32, kind="ExternalInput")
    mkd = nc.dram_tensor("mkd", [128, CPB, 2, 128], BF16, kind="ExternalInput")
    bvec = nc.dram_tensor("bvec", [128, CPB, 128], BF16, kind="ExternalInput")
    outT = nc.dram_tensor("outT", [C, NT], out_dtype, kind="ExternalOutput")
    outT_r = outT.rearrange("(f p) t -> p f t", p=128)

    with tile.TileContext(nc) as tc:
        with (
            tc.tile_pool(name="const", bufs=1) as const,
            tc.tile_pool(name="persist", bufs=1) as persist,
            tc.tile_pool(name="xp", bufs=10) as xp,
            tc.tile_pool(name="pp", bufs=10) as pp,
            tc.tile_pool(name="rp", bufs=2) as rp,
            tc.tile_pool(name="op", bufs=2) as op,
            tc.tile_pool(name="psmm", bufs=2, space="PSUM") as psmm,
            tc.tile_pool(name="psa", bufs=2, space="PSUM") as psa,
            tc.tile_pool(name="pss", bufs=2, space="PSUM") as pss,
        ):
            # ---- constants ----
            # ring order matters: the first qkv unit needs only the q-slice
            # of W plus block 0's x, so W is loaded in three f-slices (q, k,
            # v) on the gpsimd ring while the small consts + block-0 x go on
            # the sync ring.  mk/wp are queued after the first two x blocks.
            w_sb = const.tile([128, KC, F], BF16, tag="w", name="w_sb")
            nc.gpsimd.dma_start(out=w_sb[:, :, 0:128], in_=wqkv[:, :, 0:128])
            bq_sb = const.tile([128, FC], F32, tag="bq", name="bq_sb")
            nc.sync.dma_start(out=bq_sb[:], in_=bqkv[:])
            pb_sb = const.tile([128, B, NCH], F32, tag="pb", name="pb_sb")
            nc.sync.dma_start(out=pb_sb[:], in_=pbias[:])
            bv_sb = const.tile([128, CPB, 128], BF16, tag="bv", name="bv_sb")
            nc.sync.dma_start(out=bv_sb[:], in_=bvec[:])
            wp_sb = const.tile([128, C], BF16, tag="wp", name="wp_sb")
            mk_sb = const.tile([128, CPB, 2, 128], BF16, tag="mk", name="mk_sb")
            ones_sb = const.tile([128, 64], BF16, tag="ones", name="ones_sb")
            nc.vector.memset(ones_sb[:], 1.0)

            def load_wslice(which):
                nc.gpsimd.dma_start(
                    out=w_sb[:, :, which * 128:(which + 1) * 128],
                    in_=wqkv[:, :, which * 128:(which + 1) * 128])

            def load_late_consts():
                nc.sync.dma_start(out=mk_sb[:], in_=mkd[:])
                nc.sync.dma_start(out=wp_sb[:], in_=wproj[:])

            # ---- persistent per-block tiles ----
            qT = [persist.tile([128, TB], BF16, tag=f"qT{i}", name=f"qT{i}")
                  for i in range(NB)]
            kT = [persist.tile([128, TB], BF16, tag=f"kT{i}", name=f"kT{i}")
                  for i in range(NB)]
            V = [persist.tile([128, CPB, 128], BF16, tag=f"V{i}", name=f"V{i}")
                 for i in range(NB)]

            # ---- explicit PE-order chain ----
            chain = [None]

            def pe(mm):
                if chain[0] is not None:
                    add_dep_helper(mm.ins, chain[0].ins, sync=False,
                                   reason="explicit PE order")
                chain[0] = mm
                return mm

            # ---- x loads ----
            x_of = {}

            def load_group_x(g):
                # group 0 block-major: each 512-token block's 8 chunks are
                # split across the sync and gpsimd rings so the first qkv
                # unit only waits ~1MB/2rings; the W k/v slices and late
                # consts are interleaved so each arrives just before use.
                W = GRP * TB
                if g == 0:
                    for tl in range(GRP):
                        tb = g * GRP + tl
                        for kc in range(KC):
                            xt = xp.tile([128, TB], BF16, tag="xt",
                                         name="xt", bufs=32)
                            eng = nc.sync if kc < KC // 2 else nc.gpsimd
                            eng.dma_start(
                                out=xt[:],
                                in_=xT[kc * 128:(kc + 1) * 128,
                                       tb * TB:(tb + 1) * TB])
                            x_of[(tb, kc)] = xt[:]
                        if tl == 0:
                            load_wslice(1)
                            load_wslice(2)
                        elif tl == 1:
                            load_late_consts()
                else:
                    for kc in range(KC):
                        xt = xp.tile([128, W], BF16, tag="xtw", name="xtw",
                                     bufs=10)
                        nc.gpsimd.dma_start(
                            out=xt[:],
                            in_=xT[kc * 128:(kc + 1) * 128, g * W:(g + 1) * W])
                        for tl in range(GRP):
                            x_of[(g * GRP + tl, kc)] = \
                                xt[:, tl * TB:(tl + 1) * TB]

            # ---- filler units (emitted on demand, PE-chained) ----
            def qkv_unit(tb, fc):
                ps = psmm.tile([128, TB], F32, tag="ps", name="ps")
                for kc in range(KC):
                    mm = nc.tensor.matmul(
                        ps[:], lhsT=w_sb[:, kc, fc * 128:(fc + 1) * 128],
                        rhs=x_of[(tb, kc)],
                        start=(kc == 0), stop=(kc == KC - 1))
                    if kc == 0:
                        # chain only the unit head; the accumulation group
                        # orders the rest, leaving LDWEIGHTS free to pull
                        # ahead of in-flight matmuls
                        pe(mm)
                    else:
                        chain[0] = mm
                dest = qT[tb] if fc == 0 else kT[tb]
                nc.vector.tensor_scalar_add(
                    out=dest[:], in0=ps[:], scalar1=bq_sb[:, fc:fc + 1])

            def v_unit(tb):
                # V produced token-major directly: x chunk is the stationary
                # operand, V-weights stream; out[tokens, features] per chunk.
                # Avoids the DMA-transpose (xbar-mode serialization).
                psV = psmm.tile([128, CPB, 128], F32, tag="ps", name="psV")
                for cl in range(CPB):
                    for kc in range(KC):
                        mm = nc.tensor.matmul(
                            psV[:, cl, :],
                            lhsT=x_of[(tb, kc)][:, cl * 128:(cl + 1) * 128],
                            rhs=w_sb[:, kc, 2 * 128:3 * 128],
                            start=(kc == 0), stop=(kc == KC - 1),
                            skip_group_check=True)
                        if kc == 0:
                            pe(mm)
                        else:
                            chain[0] = mm
                nc.vector.tensor_add(V[tb][:], psV[:], bv_sb[:])

            def proj_mm(gb, at, stage, fc, last=False):
                # single proj matmul as a filler unit: interleaves into the
                # attention stream so the PSUM->SBUF copies (DVE/ACT) keep
                # pace with the PE instead of stalling it 4-MMs-at-a-time.
                # the final block's proj rotates through the freed psa slots
                # (psO/psD are done) -> 4-deep pipeline
                if last and fc % 2 == 1:
                    ps = psa.tile([128, TB], F32, tag="pa", name="psp")
                else:
                    ps = psmm.tile([128, TB], F32, tag="ps", name="ps")
                pe(nc.tensor.matmul(
                    ps[:], lhsT=wp_sb[:, fc * 128:(fc + 1) * 128],
                    rhs=at[:], start=True, stop=True))
                nc.any.tensor_copy(stage[:, fc, :], ps[:])
                # output DMA granularity: halves mid-kernel; quarters on the
                # last block, alternating rings, so the tail drains faster
                if last:
                    if fc % 2 == 1:
                        eng = nc.gpsimd if (fc // 2) % 2 == 0 else nc.sync
                        eng.dma_start(
                            out=outT_r[:, fc - 1:fc + 1,
                                       gb * TB:(gb + 1) * TB],
                            in_=stage[:, fc - 1:fc + 1, :])
                elif fc % (FCO // 2) == FCO // 2 - 1:
                    h0 = fc + 1 - FCO // 2
                    eng = nc.gpsimd if h0 == 0 else nc.sync
                    eng.dma_start(
                        out=outT_r[:, h0:fc + 1, gb * TB:(gb + 1) * TB],
                        in_=stage[:, h0:fc + 1, :])

            filler = []          # queue of (kind, payload) closures
            filler_pos = [0]     # how many qkv units of each group drained

            def drain_filler(n=None, through_qkv=None):
                # through_qkv: drain (incl. proj heads in the way) until every
                # qkv unit with tb <= through_qkv has been emitted
                while filler:
                    if n is not None and n <= 0:
                        break
                    kind, key, fn = filler[0]
                    if n is None and through_qkv is not None:
                        if kind == "qkv" and key > through_qkv:
                            break
                        if not any(k == "qkv" and ky <= through_qkv
                                   for k, ky, _ in filler):
                            break
                    filler.pop(0)
                    fn()
                    if n is not None:
                        n -= 1

            def push_qkv_group(g):
                load_group_x(g)
                for tl in range(GRP):
                    tb = g * GRP + tl
                    for fc in range(2):
                        filler.append(
                            ("qkv", tb,
                             lambda tb=tb, fc=fc: qkv_unit(tb, fc)))
                    filler.append(("qkv", tb, lambda tb=tb: v_unit(tb)))

            # ---- attention ----
            def s_iter(b, qb, c0, nchunks):
                """Emit S matmuls + exp (+mask) for chunks c0, c0+1."""
                gb = b * BPB + qb
                pts, offs = [], []
                for ci in (c0, c0 + 1):
                    cb, cl = divmod(ci, CPB)
                    ktile = kT[b * BPB + cb]
                    didx = ci - qb * CPB
                    off = didx * 128 if didx >= 0 else 0
                    offs.append(off)
                    psS = pss.tile([128, 2 * TB], F32, tag="pss", name="psS")
                    for h in range(HPC):
                        pe(nc.tensor.matmul(
                            psS[:, h * TB + off:(h + 1) * TB],
                            lhsT=ktile[h * 64:(h + 1) * 64,
                                       cl * 128:(cl + 1) * 128],
                            rhs=qT[gb][h * 64:(h + 1) * 64, off:],
                            start=True, stop=True,
                            tile_position=(h * 64, 0)))
                    pt = pp.tile([128, 2 * TB], BF16, tag="pt", name="pt")
                    psS3 = psS.rearrange("p (h t) -> p h t", h=2)
                    pt3 = pt.rearrange("p (h t) -> p h t", h=2)
                    nc.scalar.activation(
                        out=pt3[:, :, off:], in_=psS3[:, :, off:],
                        func=AF.Exp,
                        bias=pb_sb[:, b, ci:ci + 1], scale=scale)
                    if didx >= 0:
                        nc.gpsimd.tensor_tensor(
                            out=pt3[:, :, off:off + 128],
                            in0=pt3[:, :, off:off + 128],
                            in1=mk_sb[:, didx],
                            op=mybir.AluOpType.mult)
                    pts.append(pt)
                return pts, offs

            def pv_iter(b, qb, c0, nchunks, pts, offs, psO, psD):
                def emit_pv():
                    for j, ci in enumerate((c0, c0 + 1)):
                        vtile = V[b * BPB + ci // CPB]
                        off = offs[j]
                        for h in range(HPC):
                            pe(nc.tensor.matmul(
                                psO[h * 64:(h + 1) * 64, off:],
                                lhsT=vtile[:, ci % CPB, h * 64:(h + 1) * 64],
                                rhs=pts[j][:, h * TB + off:(h + 1) * TB],
                                start=(ci == 0), stop=(ci == nchunks - 1),
                                tile_position=(0, h * 64),
                                skip_group_check=True))

                def emit_denom():
                    for j, ci in enumerate((c0, c0 + 1)):
                        off = offs[j]
                        for h in range(HPC):
                            pe(nc.tensor.matmul(
                                psD[h * 64:(h + 1) * 64, off:],
                                lhsT=ones_sb[:],
                                rhs=pts[j][:, h * TB + off:(h + 1) * TB],
                                start=(ci == 0), stop=(ci == nchunks - 1),
                                tile_position=(0, h * 64),
                                skip_group_check=True))

                if c0 + 2 >= nchunks:
                    # last iteration of the block: denominators first so the
                    # reciprocal (gating at-mul -> proj) starts earlier
                    emit_denom()
                    emit_pv()
                else:
                    emit_pv()
                    emit_denom()

            def finish_block(b, qb, psO, psD, last=False):
                gb = b * BPB + qb
                rt = rp.tile([128, TB], F32, tag="rt", name="rt")
                nc.vector.reciprocal_approx_fast(out=rt[:], in_=psD[:])
                at = pp.tile([128, TB], BF16, tag="at", name="at", bufs=4)
                nc.vector.tensor_mul(at[:], psO[:], rt[:])
                stage = op.tile([128, FCO, TB], out_dtype, tag="st",
                                name="stage")
                for fc in range(FCO):
                    filler.append(("proj", -1,
                                   lambda fc=fc: proj_mm(gb, at, stage, fc,
                                                         last)))

            # ---- main emission: software-pipelined global stream ----
            # PV lags S by TWO iterations so the ACT exp of iteration i has
            # the PE work of iterations i+1 and i+2 (plus filler) to hide
            # behind -- with lag 1 the exp tail (~880ns vs ~850ns of PE
            # work) stalled the PV matmuls ~650ns/iteration.
            push_qkv_group(0)
            pend = []            # up to 2 of (b,qb,c0,nchunks,pts,offs,psO,psD)

            def flush_one(last=False):
                e = pend.pop(0)
                pv_iter(*e)
                if e[2] + 2 >= e[3]:         # last iter of its block
                    finish_block(e[0], e[1], e[6], e[7], last=last)

            for b in range(B):
                if b + 1 < B:
                    push_qkv_group(b + 1)
                for qb in range(BPB):
                    gb = b * BPB + qb
                    nchunks = (qb + 1) * CPB
                    # block gb needs qkv units of blocks <= gb drained
                    drain_filler(through_qkv=gb)
                    psO = psa.tile([128, TB], F32, tag="pa", name="psO")
                    psD = psa.tile([128, TB], F32, tag="pa", name="psD")
                    for c0 in range(0, nchunks, 2):
                        pts, offs = s_iter(b, qb, c0, nchunks)
                        drain_filler(n=2 if len(filler) > 16 else 1)
                        if len(pend) >= 2:
                            flush_one()
                        pend.append((b, qb, c0, nchunks, pts, offs,
                                     psO, psD))
            # flush the final two pending iterations, with filler between
            # to cover their exps
            drain_filler(n=1)
            flush_one()
            drain_filler(n=1)
            flush_one(last=True)
            drain_filler(n=len(filler))

    nc.compile()
    return nc


def prep_core_inputs(x, key_padding_mask, W_qkv, b_qkv, W_proj,
                     n_cores=8, TB=512):
    """Host-side sharding: build the per-core input maps."""
    import numpy as np
    import ml_dtypes

    B, T, C = x.shape
    D = 64
    H = C // D
    HPC = H // n_cores
    BT = B * T
    CPB = TB // 128

    xT = np.ascontiguousarray(
        x.reshape(BT, C).T).astype(ml_dtypes.bfloat16)          # [C, BT]

    pb = np.where(key_padding_mask, np.float32(-1e30),
                  np.float32(0.0)).astype(np.float32)           # [B, T]
    pb = np.ascontiguousarray(pb.reshape(B, T // 128, 128).transpose(2, 0, 1))

    # partial-mask strip for diagonal tiles: for offset idx, cols
    # [idx*128, idx*128+128) need mask (idx*128 + p <= j); replicated per head
    p = np.arange(128)[:, None]
    j = np.arange(128)[None, :]
    mk = np.stack([(p <= j) for _ in range(CPB)], axis=1)       # [128,CPB,128]
    mk = np.repeat(mk[:, :, None, :], 2, axis=2)                # [128,CPB,2,128]
    mk = mk.astype(ml_dtypes.bfloat16)

    KC = C // 128
    in_maps = []
    for c in range(n_cores):
        hs = [HPC * c + i for i in range(HPC)]
        cols = np.concatenate([
            np.concatenate([which * H * D + h * D + np.arange(D) for h in hs])
            for which in range(3)])                             # [F]
        Wc = W_qkv[:, cols]                                     # [C, F]
        F = Wc.shape[1]
        wq = np.ascontiguousarray(
            Wc.reshape(KC, 128, F).transpose(1, 0, 2)).astype(ml_dtypes.bfloat16)
        bq = np.ascontiguousarray(
            b_qkv[cols].reshape(F // 128, 128).T).astype(np.float32)
        rows = np.concatenate([h * D + np.arange(D) for h in hs])
        wp = np.ascontiguousarray(W_proj[rows, :]).astype(ml_dtypes.bfloat16)
        bv = b_qkv[cols][2 * 128:3 * 128]                       # v-bias [128]
        bvec = np.ascontiguousarray(
            np.broadcast_to(bv[None, None, :], (128, CPB, 128))
        ).astype(ml_dtypes.bfloat16)
        in_maps.append({
            "xT": xT, "wqkv": wq.reshape(128, KC, F), "wproj": wp,
            "bqkv": bq, "pbias": pb, "mkd": mk, "bvec": bvec,
        })
    return in_maps


def combine_outputs(results, B, T, C, b_proj):
    import numpy as np
    acc = results[0]["outT"].astype(np.float32)
    for r in results[1:]:
        acc = acc + r["outT"].astype(np.float32)
    out = acc.T.reshape(B, T, C) + b_proj.astype(np.float32)
    return out.astype(np.float32)


# ---------------------------------------------------------------------------
# Self-contained entry point for the grading harness.
# kernel(**inputs) takes the FULL unsharded inputs and returns the FULL output.
# Sharding: tensor-parallel over heads (2 heads per core, 8 cores); each core
# computes its QKV column-slice, attention for its heads, and a partial output
# projection; partials are summed on the host.
# ---------------------------------------------------------------------------
import numpy as np

_NC_CACHE = {}


def _get_nc():
    if "nc" not in _NC_CACHE:
        _NC_CACHE["nc"] = build_nc(B=4, T=2048, C=1024, num_devices=8)
    return _NC_CACHE["nc"]


def kernel(x, key_padding_mask, W_qkv, b_qkv, W_proj, b_proj):
    from concourse.bass_utils import run_bass_kernel_spmd

    x = np.asarray(x, dtype=np.float32)
    key_padding_mask = np.asarray(key_padding_mask).astype(bool)
    W_qkv = np.asarray(W_qkv, dtype=np.float32)
    b_qkv = np.asarray(b_qkv, dtype=np.float32)
    W_proj = np.asarray(W_proj, dtype=np.float32)
    b_proj = np.asarray(b_proj, dtype=np.float32)

    B, T, C = x.shape
    nc = _get_nc()
    in_maps = prep_core_inputs(x, key_padding_mask, W_qkv, b_qkv, W_proj,
                               n_cores=8)
    res = run_bass_kernel_spmd(nc, in_maps, list(range(8)))
    return combine_outputs(res.results, B, T, C, b_proj)

